# revision 51
# baseline (speedup 1.0000x reference)
"""Trainium2 Bass kernel for nn_GaussianActor (moe_routing).

Strategy (v2):
  - Data parallel over batch across 8 cores; weights replicated; samples
    routed by stage on host so each core gets C = sum_s ceil(n_s/8) columns
    laid out as 8 contiguous single-stage runs (zero overflow work).
  - Host folds: W3 into the per-stage heads (no activation between them),
    and the LayerNorm mean-centering into W0 (centering is a linear
    feature-space projection: W0'' = W0 - colmean(W0), b0'' = b0 - mean(b0)),
    so the device only computes the variance.
  - All matmuls fp16 (full PE rate, LDWEIGHTS FWL, 2x DVE/DMA); fp32 PSUM.
  - Variance: DVE squares + gpsimd pairwise tree + one ones-reduce matmul;
    rstd broadcast via a [1,128]-stationary matmul of the variance row, then
    Newton rsqrt iterations on DVE in the broadcast [128,cw] domain (no
    scalar-engine table functions anywhere except Lrelu).
  - Scalar engine runs ONLY Lrelu activations (LN apply incl gamma/beta,
    L1/L2 bias+lrelu evictions) -> single activation table load.
  - Head: per (chunk x stage-run) piece matmul chains, single head each.
  - Software pipeline: chunk i+1's L0/stats interleaved inside chunk i's
    L1/L2/head so the tensor engine never waits on LN statistics.
"""

import numpy as np

import concourse.tile as tile
from concourse import bacc, mybir
from concourse import bass_utils
from concourse.alu_op_type import AluOpType

dt = mybir.dt
AF = mybir.ActivationFunctionType

B = 32768
OBS = 512
HID = 1024
A2 = 128          # 2 * action_dim
NSTAGE = 8
NCORES = 8

CH = 512          # chunk width (PSUM bank limit: 512 fp32 per matmul)
KO = OBS // 128   # 4 k-blocks for layer 0
KH = HID // 128   # 8 k-blocks for hidden layers
MH = HID // 128   # 8 m-blocks of hidden features

EPS = 1e-5
SLOPE = 0.01
LOG_STD_MIN, LOG_STD_MAX = -20.0, 2.0

_CACHE = {}


def _build_nc(C, pieces_per_chunk, chunk_widths, ln_trivial=True):
    """pieces_per_chunk: list (per chunk) of (stage, a, b) column pieces
    relative to the chunk start. chunk_widths: list of chunk widths.
    ln_trivial: ln_w all-ones and ln_b all-zeros (skip affine on DVE path)."""
    nc = bacc.Bacc("TRN2", target_bir_lowering=False, debug=False,
                   num_devices=NCORES)

    # all inputs packed partition-major ([128, k*cols]) so DMA lines are
    # 4-16KB contiguous per partition -> full DGE packet throughput
    obsX = nc.dram_tensor("obsX", [128, KO * C], dt.float16,
                          kind="ExternalInput").ap()
    w0 = nc.dram_tensor("w0", [128, KO * HID], dt.float16,
                        kind="ExternalInput").ap()
    w1 = nc.dram_tensor("w1", [128, KH * HID], dt.float16,
                        kind="ExternalInput").ap()
    w2 = nc.dram_tensor("w2", [128, KH * HID], dt.float16,
                        kind="ExternalInput").ap()
    wh = nc.dram_tensor("wh", [128, KH * NSTAGE * A2], dt.float16,
                        kind="ExternalInput").ap()
    # fp32 per-partition vectors: cols 0:8 b0c, 8:16 b1, 16:24 b2,
    # 24:32 bhat (per stage), 32:40 ln beta, 40:48 ln gamma
    bias = nc.dram_tensor("bias", [128, 48], dt.float32, kind="ExternalInput").ap()
    onesk = nc.dram_tensor("onesk", [128, 1], dt.float16, kind="ExternalInput").ap()
    ones2 = nc.dram_tensor("ones2", [1, 128], dt.float16, kind="ExternalInput").ap()

    out = nc.dram_tensor("out", [A2, C], dt.float32, kind="ExternalOutput").ap()

    NCH = len(chunk_widths)
    chunk_off = np.concatenate([[0], np.cumsum(chunk_widths)]).astype(int)

    with tile.TileContext(nc) as tc:
        with tc.tile_pool(name="w", bufs=1) as wp, \
             tc.tile_pool(name="acts", bufs=1) as ap_, \
             tc.tile_pool(name="pm", bufs=6, space="PSUM") as pm, \
             tc.tile_pool(name="pr", bufs=1, space="PSUM") as pr, \
             tc.tile_pool(name="pb", bufs=1, space="PSUM") as pb:

            st = {}

            def _xdma(i, half, eng):
                """DMA one k-half (k=0,1 or k=2,3) of chunk i's packed x."""
                cw = int(chunk_widths[i])
                c0 = int(chunk_off[i])
                if ("x", i) not in st:
                    st[("x", i)] = ap_.tile([128, KO * CH], dt.float16,
                                            tag="x", bufs=4, name=f"x_{i}")
                xt = st[("x", i)]
                a = half * 2 * cw
                eng.dma_start(xt[:, a:a + 2 * cw],
                              obsX[:, KO * c0 + a:KO * c0 + a + 2 * cw])

            def emit_xdma(i):
                _xdma(i, 0, nc.sync)
                _xdma(i, 1, nc.gpsimd)

            def emit_L0(i, mid=None):
                """L0 matmuls, h' evict (ACT Identity), squares+tree (DVE).
                mid: optional {m: callback} to interleave tensor work."""
                cw = int(chunk_widths[i])
                c0 = int(chunk_off[i])
                xt = st.pop(("x", i))
                hp = []
                sq = []
                for m in range(MH):
                    p = pm.tile([128, CH], dt.float32, tag="pm", bufs=6,
                                name=f"p0_{i}_{m}")
                    for k in range(KO):
                        nc.tensor.matmul(p[:, :cw],
                                         w0t[:, k * HID + m * 128:
                                             k * HID + (m + 1) * 128],
                                         xt[:, k * cw:(k + 1) * cw],
                                         start=(k == 0), stop=(k == KO - 1))
                    h = ap_.tile([128, CH], dt.float16, tag="hp", bufs=20,
                                 name=f"h_{i}_{m}")
                    nc.scalar.activation(h[:, :cw], p[:, :cw], AF.Identity,
                                         bias=biast[:, m:m + 1], scale=1.0)
                    hp.append(h)
                    s = ap_.tile([128, CH], dt.float16, tag="sq", bufs=12,
                                 name=f"sq_{i}_{m}")
                    nc.vector.tensor_tensor(s[:, :cw], h[:, :cw], h[:, :cw],
                                            AluOpType.mult)
                    sq.append(s)
                    if mid and m in mid:
                        mid[m]()
                lvl = sq
                li = 0
                while len(lvl) > 1:
                    nxt = []
                    for j in range(0, len(lvl), 2):
                        o = ap_.tile([128, CH], dt.float16, tag="sq", bufs=12,
                                     name=f"tr_{i}_{li}_{j}")
                        nc.vector.tensor_tensor(o[:, :cw], lvl[j][:, :cw],
                                                lvl[j + 1][:, :cw],
                                                AluOpType.add)
                        nxt.append(o)
                    lvl = nxt
                    li += 1
                return dict(i=i, cw=cw, c0=c0, hp=hp, S=lvl[0])

            def emit_var(cur):
                """variance ones-reduce matmul + row evict (DVE)."""
                i, cw = cur["i"], cur["cw"]
                pv = pr.tile([1, CH], dt.float32, tag="pr", name=f"pv_{i}")
                nc.tensor.matmul(pv[:, :cw], oneskt[:], cur["S"][:, :cw],
                                 start=True, stop=True)
                row = ap_.tile([1, CH], dt.float16, tag="row", bufs=2,
                               name=f"row_{i}")
                nc.vector.tensor_scalar(row[:, :cw], pv[:, :cw],
                                        1.0 / HID, None, AluOpType.mult)
                cur["row"] = row

            def emit_bcast_ln(cur):
                """bcast matmul, Newton rsqrt (DVE), LN mults (DVE),
                z0 m0..3 lrelu via DVE max-trick."""
                i, cw, hp = cur["i"], cur["cw"], cur["hp"]
                pvb = pb.tile([128, CH], dt.float32, tag="pb", name=f"pvb_{i}")
                nc.tensor.matmul(pvb[:, :cw], ones2t[:], cur["row"][:1, :cw],
                                 start=True, stop=True)
                # t0 = 3*(v + eps)  (fp32, reads psum)
                t0 = ap_.tile([128, CH], dt.float32, tag="nt0", bufs=2,
                              name=f"nt0_{i}")
                nc.vector.tensor_scalar(t0[:, :cw], pvb[:, :cw],
                                        EPS, 3.0,
                                        AluOpType.add, AluOpType.mult)
                # y1 = 2.5980762 - 0.8660254*t0   (y0 = sqrt(3))
                y = ap_.tile([128, CH], dt.float16, tag="ny", bufs=4,
                             name=f"ny1_{i}")
                nc.vector.tensor_scalar(y[:, :cw], t0[:, :cw],
                                        -0.8660254, 2.5980762,
                                        AluOpType.mult, AluOpType.add)
                for it in range(2):
                    s = ap_.tile([128, CH], dt.float16, tag="ns", bufs=2,
                                 name=f"ns_{i}_{it}")
                    nc.vector.tensor_tensor(s[:, :cw], y[:, :cw], y[:, :cw],
                                            AluOpType.mult)
                    u = ap_.tile([128, CH], dt.float16, tag="nu", bufs=2,
                                 name=f"nu_{i}_{it}")
                    nc.vector.tensor_tensor(u[:, :cw], t0[:, :cw], s[:, :cw],
                                            AluOpType.mult)
                    w_ = ap_.tile([128, CH], dt.float16, tag="nw", bufs=2,
                                  name=f"nw_{i}_{it}")
                    nc.vector.tensor_scalar(w_[:, :cw], u[:, :cw],
                                            -1.0 / 6.0, 1.5,
                                            AluOpType.mult, AluOpType.add)
                    y2 = ap_.tile([128, CH], dt.float16, tag="ny", bufs=4,
                                  name=f"ny_{i}_{it}")
                    nc.vector.tensor_tensor(y2[:, :cw], y[:, :cw], w_[:, :cw],
                                            AluOpType.mult)
                    y = y2
                rb = y
                # LN multiplies for all m (DVE); z0 for m0..3 finished on DVE
                z0 = [None] * MH
                lts = []
                for m in range(MH):
                    t = ap_.tile([128, CH], dt.float16, tag="lt", bufs=10,
                                 name=f"lt_{i}_{m}")
                    # chunk 0 only (cold open): m4-7 multiplies on the then-
                    # idle gpsimd engine, parallel to the DVE Newton burst
                    eng = nc.gpsimd if (i == 0 and m >= 4) else nc.vector
                    eng.tensor_tensor(t[:, :cw], hp[m][:, :cw],
                                      rb[:, :cw], AluOpType.mult)
                    lts.append(t)
                    if m < 4:
                        u = t
                        if not ln_trivial:
                            u = ap_.tile([128, CH], dt.float16, tag="lu",
                                         bufs=4, name=f"lu_{i}_{m}")
                            nc.vector.tensor_scalar(
                                u[:, :cw], t[:, :cw],
                                biast[:, 40 + m:41 + m],
                                biast[:, 32 + m:33 + m],
                                AluOpType.mult, AluOpType.add)
                        z = ap_.tile([128, CH], dt.float16, tag="z0", bufs=12,
                                     name=f"z0_{i}_{m}")
                        nc.vector.scalar_tensor_tensor(
                            z[:, :cw], u[:, :cw], SLOPE, u[:, :cw],
                            AluOpType.mult, AluOpType.max)
                        z0[m] = z
                cur["lts"] = lts
                cur["z0"] = z0

            def emit_ln_act(cur):
                """z0 m4..7 via ACT Lrelu (late ACT-queue slot)."""
                i, cw = cur["i"], cur["cw"]
                for m in range(4, MH):
                    z = ap_.tile([128, CH], dt.float16, tag="z0", bufs=12,
                                 name=f"z0_{i}_{m}")
                    nc.scalar.activation(z[:, :cw], cur["lts"][m][:, :cw],
                                         AF.Lrelu,
                                         bias=biast[:, 32 + m:33 + m],
                                         scale=biast[:, 40 + m:41 + m],
                                         alpha=SLOPE)
                    cur["z0"][m] = z

            def emit_L1(cur, ms):
                i, cw = cur["i"], cur["cw"]
                w1t = st["w"][0]
                z0 = cur["z0"]
                z1 = cur.setdefault("z1", [])
                for m in ms:
                    p = pm.tile([128, CH], dt.float32, tag="pm", bufs=6,
                                name=f"p1_{i}_{m}")
                    for k in range(KH):
                        nc.tensor.matmul(p[:, :cw],
                                         w1t[:, k * HID + m * 128:
                                             k * HID + (m + 1) * 128],
                                         z0[k][:, :cw],
                                         start=(k == 0), stop=(k == KH - 1))
                    z = ap_.tile([128, CH], dt.float16, tag="z1", bufs=10,
                                 name=f"z1_{i}_{m}")
                    nc.scalar.activation(z[:, :cw], p[:, :cw], AF.Lrelu,
                                         bias=biast[:, 8 + m:9 + m],
                                         scale=1.0, alpha=SLOPE)
                    z1.append(z)

            def emit_L2(cur, ms):
                i, cw = cur["i"], cur["cw"]
                w2t = st["w"][1]
                z1 = cur["z1"]
                z2 = cur.setdefault("z2", [])
                for m in ms:
                    p = pm.tile([128, CH], dt.float32, tag="pm", bufs=6,
                                name=f"p2_{i}_{m}")
                    for k in range(KH):
                        nc.tensor.matmul(p[:, :cw],
                                         w2t[:, k * HID + m * 128:
                                             k * HID + (m + 1) * 128],
                                         z1[k][:, :cw],
                                         start=(k == 0), stop=(k == KH - 1))
                    z = ap_.tile([128, CH], dt.float16, tag="z2", bufs=10,
                                 name=f"z2_{i}_{m}")
                    nc.scalar.activation(z[:, :cw], p[:, :cw], AF.Lrelu,
                                         bias=biast[:, 16 + m:17 + m],
                                         scale=1.0, alpha=SLOPE)
                    z2.append(z)

            def emit_head(cur):
                """head pieces + eviction (DVE) + per-piece output DMA."""
                i, cw, c0 = cur["i"], cur["cw"], cur["c0"]
                wht = st["w"][2]
                z2 = cur["z2"]
                ph = pm.tile([128, CH], dt.float32, tag="pm", bufs=6,
                             name=f"ph_{i}")
                ot = ap_.tile([128, CH], dt.float32, tag="ot", bufs=3,
                              name=f"ot_{i}")
                SA = NSTAGE * A2
                last = (i == NCH - 1)
                for (s, a, b) in pieces_per_chunk[i]:
                    for k in range(KH):
                        nc.tensor.matmul(ph[:, a:b],
                                         wht[:, k * SA + s * A2:
                                             k * SA + (s + 1) * A2],
                                         z2[k][:, a:b],
                                         start=(k == 0), stop=(k == KH - 1))
                    # final chunk: halve evict+DMA so the tail DMA starts
                    # as soon as possible after the last matmul
                    cuts = ([a, (a + b) // 2, b] if last and b - a >= 64
                            else [a, b])
                    for aa, bb in zip(cuts, cuts[1:]):
                        nc.vector.tensor_scalar_add(ot[:, aa:bb], ph[:, aa:bb],
                                                    biast[:, 24 + s:25 + s])
                        nc.scalar.dma_start(out[:, c0 + aa:c0 + bb],
                                            ot[:, aa:bb])

            # ---- software pipeline ----
            # tensor order per period i (L0 runs TWO chunks ahead so deep
            # weights get extra landing time and LN latency is fully hidden):
            #   L0(i+2) | L1(i) m01 | var(i+1) | L1 m23 | bcast(i+1) |
            #   L1 m4-7 | L2(i) | head(i)
            # PE warm-up: dummy matmuls on a memset tile cover the ~10us DMA
            # channel startup so real matmuls start at HAM K=8/8.
            zmm = ap_.tile([128, 256], dt.float16, tag="zmm", bufs=1,
                           name="zmm")
            nc.vector.memset(zmm[:], 0.0)
            pdum = pb.tile([128, CH], dt.float32, tag="pb", name="pdum")
            for j in range(54):
                nc.tensor.matmul(pdum[:, :256], zmm[:, :128], zmm[:],
                                 start=True, stop=True)

            # prologue DMAs: hand-ordered per channel so every tensor lands
            # just before first use. sync + scalar are HW DGE (fast);
            # gpsimd is software DGE (slow) and gets the laggards. Each
            # weight is one wide tile (8-16KB/partition lines, split in two
            # half-DMAs across the channels).
            w0t_ = wp.tile([128, KO * HID], dt.float16, tag="w0", name="w0t")
            w0t = w0t_
            w1t = wp.tile([128, KH * HID], dt.float16, tag="w1", name="w1t")
            w2t = wp.tile([128, KH * HID], dt.float16, tag="w2", name="w2t")
            wht = wp.tile([128, KH * NSTAGE * A2], dt.float16, tag="wh",
                          name="wht")
            biast = wp.tile([128, 48], dt.float32, tag="bias", name="bias")
            oneskt = wp.tile([128, 1], dt.float16, tag="onesk", name="onesk")
            ones2t = wp.tile([1, 128], dt.float16, tag="ones2", name="ones2")
            st["w"] = (w1t, w2t, wht)

            def _wh(eng, t, src, half, n):
                a, b = half * n // 2, (half + 1) * n // 2
                eng.dma_start(t[:, a:b], src[:, a:b])

            # sync channel: w0/x0 halves, then w1 (early!), x2, w2, wh
            _wh(nc.sync, w0t_, w0, 0, KO * HID)
            _xdma(0, 0, nc.sync)
            nc.sync.dma_start(biast[:], bias[:])
            nc.sync.dma_start(oneskt[:], onesk[:])
            _wh(nc.sync, w1t, w1, 0, KH * HID)
            if NCH > 2:
                _xdma(2, 0, nc.sync)
            _wh(nc.sync, w2t, w2, 0, KH * HID)
            _wh(nc.sync, wht, wh, 0, KH * NSTAGE * A2)
            # scalar channel (issues on ACT queue, before any ACT compute)
            _wh(nc.scalar, w0t_, w0, 1, KO * HID)
            _xdma(0, 1, nc.scalar)
            _wh(nc.scalar, w1t, w1, 1, KH * HID)
            if NCH > 2:
                _xdma(2, 1, nc.scalar)
            _wh(nc.scalar, w2t, w2, 1, KH * HID)
            _wh(nc.scalar, wht, wh, 1, KH * NSTAGE * A2)
            # gpsimd channel (slow SW DGE): constants and x1 only
            nc.gpsimd.dma_start(ones2t[:], ones2[:])
            if NCH > 1:
                _xdma(1, 0, nc.gpsimd)
                _xdma(1, 1, nc.gpsimd)

            chk = [None] * (NCH + 2)
            chk[0] = emit_L0(0)
            if NCH > 1:
                c0_ = chk[0]
                chk[1] = emit_L0(1, mid={3: lambda: emit_var(c0_),
                                         5: lambda: emit_bcast_ln(c0_)})
                emit_ln_act(chk[0])
            else:
                emit_var(chk[0])
                emit_bcast_ln(chk[0])
                emit_ln_act(chk[0])
            for i in range(NCH):
                cur = chk[i]
                if i + 2 < NCH:
                    chk[i + 2] = emit_L0(i + 2)
                if i + 3 < NCH:
                    emit_xdma(i + 3)
                nxt = chk[i + 1] if i + 1 < NCH else None
                emit_L1(cur, [0, 1])
                if nxt is not None:
                    emit_var(nxt)
                emit_L1(cur, [2, 3])
                if nxt is not None:
                    emit_bcast_ln(nxt)
                emit_L1(cur, [4, 5, 6, 7])
                emit_L2(cur, range(MH))
                if nxt is not None:
                    emit_ln_act(nxt)
                emit_head(cur)

    nc.compile()
    return nc


def _layout(stage):
    """Static run layout from stage counts: per-core widths w_s (even),
    chunk grid, and head pieces per chunk."""
    n = np.bincount(stage, minlength=NSTAGE)
    w = ((n + 2 * NCORES - 1) // (2 * NCORES)) * 2     # ceil(n_s/8) -> even
    C = int(w.sum())
    R = np.concatenate([[0], np.cumsum(w)]).astype(int)
    # chunk widths: 512-grid with a possibly short last chunk
    nch = (C + CH - 1) // CH
    chunk_widths = [CH] * (nch - 1) + [C - CH * (nch - 1)]
    chunk_off = np.concatenate([[0], np.cumsum(chunk_widths)]).astype(int)
    pieces = []
    for i in range(nch):
        c0, c1 = int(chunk_off[i]), int(chunk_off[i + 1])
        pc = []
        for s in range(NSTAGE):
            a, b = max(c0, int(R[s])), min(c1, int(R[s + 1]))
            if a < b:
                pc.append((s, a - c0, b - c0))
        pieces.append(pc)
    return n, w, C, R, chunk_widths, pieces


def _get_nc(C, chunk_widths, pieces, ln_trivial):
    key = (C, tuple(chunk_widths), ln_trivial,
           tuple(tuple(p) for pc in pieces for p in pc))
    if key not in _CACHE:
        _CACHE[key] = _build_nc(C, pieces, chunk_widths, ln_trivial)
    return _CACHE[key]


def _prep(inputs):
    obs = np.asarray(inputs["obs"], np.float32)
    stage = np.asarray(inputs["stage"]).astype(np.int64)
    W0 = np.asarray(inputs["W0"], np.float64)
    b0 = np.asarray(inputs["b0"], np.float64)
    ln_w = np.asarray(inputs["ln_w"], np.float32)
    ln_b = np.asarray(inputs["ln_b"], np.float32)
    W1 = np.asarray(inputs["W1"], np.float32)
    b1 = np.asarray(inputs["b1"], np.float32)
    W2 = np.asarray(inputs["W2"], np.float32)
    b2 = np.asarray(inputs["b2"], np.float32)
    W3 = np.asarray(inputs["W3"], np.float32)
    b3 = np.asarray(inputs["b3"], np.float32)
    Wh = np.asarray(inputs["Wh"], np.float32)
    bh = np.asarray(inputs["bh"], np.float32)

    n, w, C, R, chunk_widths, pieces = _layout(stage)

    # fold mean-centering into W0 / b0
    W0c = (W0 - W0.mean(axis=1, keepdims=True)).astype(np.float16)
    b0c = (b0 - b0.mean()).astype(np.float32)
    # fold W3 into heads
    What = np.einsum("kj,sjo->sko", W3, Wh)            # [S, HID, A2]
    whcat = np.concatenate([What[s] for s in range(NSTAGE)],
                           axis=1).astype(np.float16)  # [HID, S*A2]
    bhat = (b3[None, :] @ Wh)[:, 0, :] + bh            # [S, A2]

    bias = np.zeros((128, 48), np.float32)
    bias[:, 0:8] = b0c.reshape(MH, 128).T
    bias[:, 8:16] = b1.reshape(MH, 128).T
    bias[:, 16:24] = b2.reshape(MH, 128).T
    bias[:, 24:32] = bhat.T.astype(np.float32)         # [A2, S]
    bias[:, 32:40] = ln_b.reshape(MH, 128).T
    bias[:, 40:48] = ln_w.reshape(MH, 128).T

    def pack_kmajor(W):
        """[K*128, N] -> [128, K*N] so DMA lines are contiguous/partition."""
        K = W.shape[0] // 128
        return np.ascontiguousarray(
            W.reshape(K, 128, -1).transpose(1, 0, 2).reshape(128, -1))

    shared = {
        "w0": pack_kmajor(W0c),
        "w1": pack_kmajor(W1.astype(np.float16)),
        "w2": pack_kmajor(W2.astype(np.float16)),
        "wh": pack_kmajor(whcat),
        "bias": bias,
        "onesk": np.ones((128, 1), np.float16),
        "ones2": np.ones((1, 128), np.float16),
    }

    chunk_off = np.concatenate([[0], np.cumsum(chunk_widths)]).astype(int)
    # route: per stage, sorted sample ids; core c takes slice [c*w_s,(c+1)*w_s)
    order = [np.where(stage == s)[0] for s in range(NSTAGE)]
    obsT16 = np.ascontiguousarray(obs.T.astype(np.float16))   # [OBS, B]
    in_maps, perms = [], []
    for c in range(NCORES):
        perm = np.zeros(C, np.int64)
        for s in range(NSTAGE):
            lo = min(c * w[s], n[s])
            hi = min((c + 1) * w[s], n[s])
            seg = order[s][lo:hi]
            cols = np.arange(R[s], R[s] + (hi - lo))
            perm[cols] = seg
            # pad columns keep sample 0 (value irrelevant, discarded)
        m = dict(shared)
        oc = obsT16[:, perm]                       # [OBS, C]
        xparts = []
        for i, cw_ in enumerate(chunk_widths):
            c0 = int(chunk_off[i])
            seg = oc[:, c0:c0 + int(cw_)]          # [OBS, cw]
            xparts.append(seg.reshape(KO, 128, int(cw_))
                          .transpose(1, 0, 2).reshape(128, -1))
        m["obsX"] = np.ascontiguousarray(np.concatenate(xparts, axis=1))
        in_maps.append(m)
        perms.append(perm)
    return in_maps, perms, (n, w, C, R, chunk_widths, pieces)


def _unpack(results, perms, layout):
    n, w, C, R, chunk_widths, pieces = layout
    out = np.zeros((B, A2), np.float32)
    for c in range(NCORES):
        oc = results[c]["out"]                         # [A2, C]
        for s in range(NSTAGE):
            lo = min(c * w[s], n[s])
            hi = min((c + 1) * w[s], n[s])
            if hi > lo:
                idx = perms[c][R[s]:R[s] + (hi - lo)]
                out[idx] = oc[:, R[s]:R[s] + (hi - lo)].T
    return out


def _run(inputs, trace=False, tmpdir=None):
    in_maps, perms, layout = _prep(inputs)
    n, w, C, R, chunk_widths, pieces = layout
    ln_trivial = bool(np.all(np.asarray(inputs["ln_w"]) == 1.0)
                      and np.all(np.asarray(inputs["ln_b"]) == 0.0))
    nc = _get_nc(C, chunk_widths, pieces, ln_trivial)
    res = bass_utils.run_bass_kernel_spmd(nc, in_maps, list(range(NCORES)),
                                          trace=trace, tmpdir=tmpdir)
    out = _unpack(res.results, perms, layout)
    mean = np.ascontiguousarray(out[:, :64])
    log_std = np.clip(out[:, 64:], LOG_STD_MIN, LOG_STD_MAX)
    return (mean, log_std), res


def kernel(**inputs):
    (mean, log_std), _ = _run(inputs, trace=False)
    return mean, log_std


def kernel_timed(_tmpdir=None, **inputs):
    (mean, log_std), res = _run(inputs, trace=True, tmpdir=_tmpdir)
    return (mean, log_std), res
